# revision 63
# baseline (speedup 1.0000x reference)
"""Trainium2 Bass kernel for nn_Attention_47682726920277.

Causal multi-head attention with RoPE:
  q/k/v = x @ w{q,k,v}.T ; RoPE(q, k) ; att = softmax(mask(q k^T / 8)) ; out = (att v) @ wo.T
Shapes: x [2, 2048, 1024], 16 heads of dim 64, fp32.

Sharding (8 cores): data-parallel over batch (2) x tensor-parallel over heads (4 per
core). Each core computes its 4 heads' attention and a partial out via its wo row
block; the final all-reduce is the host-side sum of the 4 partials per batch.

Per-core design (v2 — software-pipelined):
  - Q,K produced transposed (QT/KT [256ch, T] fp16) so scores come out as S^T [k, q];
    V is augmented with a ones column so row 64 of the PV accumulator is the softmax
    denominator.  Exp runs on the scalar engine with the 1/8 scale fused; the scalar
    engine runs *only* exp (one activation table, loaded once).
  - RoPE runs at the DVE 2x fp16 rate: one cast from PSUM, a partition-XOR-32 swap
    done by 4 small SBUF->SBUF DMAs, then 3 full-width fp16 multiplies/adds.
  - Normalization: DVE reciprocal of the sums row, gpsimd partition-broadcast, one
    DVE multiply into attnT.  No DRAM roundtrip, no Ln on the scalar engine.
  - PSUM drains via direct DMA (PSUM->SBUF staging, PSUM->DRAM for the wo output)
    instead of vector-engine copies.
  - Emission interleaves the two head-pairs' attention per q-chunk and feeds the
    tensor engine projection/wo matmuls as fillers *inside* the attention kb-loop,
    so the PE never idles waiting on exp and the HAM clock gate stays at 2.4 GHz.
"""
import sys
import types
import numpy as np

B = 2
T = 2048
D = 1024
H = 16
HD = 64
NCORES = 8
GROUPS = NCORES // B          # head-groups per batch
HPC = H // GROUPS             # heads per core = 4
CH = HPC * HD                 # channels per core = 256
NQ = 512                      # PSUM bank width (fp32)
P = 128

_prog_cache = {}


def _install_ntff_shim():
    """The agent image's antenv lacks axon_hooks; inject it so trace=True works."""
    try:
        import antenv.axon_hooks  # noqa: F401
        return
    except ImportError:
        pass
    try:
        import trn_agent_boot.trn_boot as tb
        hook = tb._ntff_profile_via_ctypes('/opt/axon/libaxon_pjrt.so')
        if hook is None:
            return
        mod = types.ModuleType('antenv.axon_hooks')
        mod.get_axon_ntff_profile_hook = lambda: hook
        mod.set_axon_ntff_profile_hook = lambda h: None
        sys.modules['antenv.axon_hooks'] = mod
        import antenv
        antenv.axon_hooks = mod
    except Exception:
        pass


def _build_program(causal: bool):
    import concourse.bass as bass
    from concourse import bacc
    import concourse.tile as tile
    from concourse import mybir

    F32 = mybir.dt.float32
    F16 = mybir.dt.float16
    AF = mybir.ActivationFunctionType
    MUL = mybir.AluOpType.mult
    ADD = mybir.AluOpType.add

    NT = T // NQ          # proj/attention q-chunks (4)
    NKB = T // P          # k-blocks (16)
    DB = D // P           # d-blocks (8)
    CB = CH // P          # channel blocks = head-pair blocks (2)

    nc = bacc.Bacc("TRN2", target_bir_lowering=False, debug=False)

    xT = nc.dram_tensor("xT", [D, T], F16, kind="ExternalInput").ap()
    wqT = nc.dram_tensor("wqT", [D, CH], F16, kind="ExternalInput").ap()
    wkT = nc.dram_tensor("wkT", [D, CH], F16, kind="ExternalInput").ap()
    wvT = nc.dram_tensor("wvT", [D, CH], F16, kind="ExternalInput").ap()
    woT = nc.dram_tensor("woT", [CH, D], F16, kind="ExternalInput").ap()
    cosS = nc.dram_tensor("cosS", [P, T], F16, kind="ExternalInput").ap()
    sinS = nc.dram_tensor("sinS", [P, T], F16, kind="ExternalInput").ap()
    ident = nc.dram_tensor("ident", [P, P], F16, kind="ExternalInput").ap()
    triB = nc.dram_tensor("triB", [P, P], F16, kind="ExternalInput").ap()
    onescol = nc.dram_tensor("onescol", [P, NKB * HPC], F16, kind="ExternalInput").ap()
    out = nc.dram_tensor("out", [T, D], F16, kind="ExternalOutput").ap()

    with tile.TileContext(nc) as tc:
        with tc.tile_pool(name="singles", bufs=1) as singles, \
             tc.tile_pool(name="rope16", bufs=3) as rope16, \
             tc.tile_pool(name="ptp", bufs=3) as ptp, \
             tc.tile_pool(name="obp", bufs=3) as obp, \
             tc.tile_pool(name="ssm", bufs=2) as ssm, \
             tc.tile_pool(name="bcp", bufs=4) as bcp, \
             tc.tile_pool(name="dramp", bufs=1, space="DRAM") as dramp, \
             tc.tile_pool(name="pspool", bufs=2, space="PSUM") as pspool:

            # ---- resident tiles ----
            xT_sb = singles.tile([P, DB, T], F16)
            wqT_sb = singles.tile([P, DB, CH], F16)
            wkT_sb = singles.tile([P, DB, CH], F16)
            wvT_sb = singles.tile([P, DB, CH], F16)
            woT_sb = singles.tile([P, CB, D], F16)
            cosS_sb = singles.tile([P, T], F16)
            sinS_sb = singles.tile([P, T], F16)
            ident_sb = singles.tile([P, P], F16)
            triB_sb = singles.tile([P, P], F16)
            QT_sb = singles.tile([P, CB, T], F16)
            KT_sb = singles.tile([P, CB, T], F16)
            attnT_sb = singles.tile([P, CB, T], F16)
            # V with a ones column per head: [kb, head, 65]
            vaug = singles.tile([P, NKB, HPC, HD + 1], F16)
            # staged unnormalized attention outputs [65, hp, qc*2+half, q]
            otsb = singles.tile([HD + 1, CB, 2 * NT, NQ], F32)
            sumd = dramp.tile([NT, 4, NQ], F32)
            recd = dramp.tile([NT, 4, NQ], F32, name="recd")

            xTr = xT.rearrange("(o p) t -> p o t", p=P)

            # ---- resident loads, ordered so the pipeline can start ASAP ----
            nc.sync.dma_start(ident_sb[:], ident[:])
            nc.sync.dma_start(wqT_sb[:], wqT.rearrange("(o p) c -> p o c", p=P))
            for o in range(DB):
                nc.sync.dma_start(xT_sb[:, o, 0:NQ], xTr[:, o, 0:NQ])
            nc.sync.dma_start(wkT_sb[:], wkT.rearrange("(o p) c -> p o c", p=P))
            nc.sync.dma_start(cosS_sb[:], cosS[:])
            nc.sync.dma_start(sinS_sb[:], sinS[:])
            nc.sync.dma_start(triB_sb[:], triB[:])

            # warm-up burst: ~3.5us of dummy matmuls on the identity tile
            # while the first weight/activation DMAs land, so the HAM clock
            # gate releases before the real projections start
            wup = pspool.tile([P, 2, NQ], F32, tag="st", name="warmup")
            for w in range(35):
                nc.tensor.matmul(wup[:, 0, 0:P], ident_sb[:], ident_sb[:],
                                 start=True, stop=True)
            nc.sync.dma_start(wvT_sb[:], wvT.rearrange("(o p) c -> p o c", p=P))
            nc.sync.dma_start(
                vaug[:, :, :, HD:HD + 1],
                onescol.rearrange("p (a b) -> p a b", a=NKB)[:, :, :, None])
            for o in range(DB):
                nc.sync.dma_start(xT_sb[:, o, NQ:T], xTr[:, o, NQ:T])
            nc.sync.dma_start(woT_sb[:], woT.rearrange("(o p) c -> p o c", p=P))

            # ---------------- unit emitters (filler work) ----------------
            # Each unit is a list of closures; each closure emits ~1 PE matmul
            # (plus trailing cheap ops).  The attention kb-loop pops one step
            # per iteration so the PE always has independent fill work.

            def proj_unit(w_sb, dst_sb, cb, m, pname):
                """q/k projection of one 512-chunk + fp16 RoPE. 8 MM steps + tail."""
                cs = slice(m * NQ, (m + 1) * NQ)
                state = {}

                def mk_mm(o):
                    def step():
                        if o == 0:
                            state['ps'] = pspool.tile(
                                [P, NQ], F32, tag="b1", bufs=4,
                                name=f"prj_{pname}_{cb}_{m}")
                        nc.tensor.matmul(
                            state['ps'][:],
                            w_sb[:, o, cb * P:(cb + 1) * P],
                            xT_sb[:, o, cs],
                            start=(o == 0), stop=(o == DB - 1))
                    return step

                def tail():
                    ps = state['ps']
                    qraw = rope16.tile([P, NQ], F16, tag="qraw",
                                       name=f"qr_{pname}_{cb}_{m}")
                    nc.vector.tensor_copy(qraw[:], ps[:])
                    # q' = q*cos + swap32(q)*sin  (sign folded into sin table)
                    nc.vector.tensor_tensor(dst_sb[:, cb, cs], qraw[:],
                                            cosS_sb[:, cs], MUL)
                    qswp = rope16.tile([P, NQ], F16, tag="qswp",
                                       name=f"qs_{pname}_{cb}_{m}")
                    for g in range(4):
                        src = (g ^ 1) * 32
                        dst = g * 32
                        # 32-aligned partition-base shift; fp16 copy runs in
                        # the DVE 4x packed mode (~194ns each)
                        nc.vector.tensor_copy(qswp[dst:dst + 32, :],
                                              qraw[src:src + 32, :])
                    tmp = rope16.tile([P, NQ], F16, tag="tmp",
                                      name=f"tm_{pname}_{cb}_{m}")
                    nc.vector.tensor_tensor(tmp[:], qswp[:], sinS_sb[:, cs], MUL)
                    nc.vector.tensor_tensor(dst_sb[:, cb, cs],
                                            dst_sb[:, cb, cs], tmp[:], ADD)

                return [mk_mm(o) for o in range(DB)] + [tail]

            def vproj_unit(i):
                """V projection of one 128-row t-block: 8 MM steps + cast tail."""
                state = {}

                def mk_mm(o):
                    def step():
                        if o == 0:
                            state['ps'] = pspool.tile(
                                [P, NQ], F32, tag="b1", bufs=4, name=f"v_{i}")
                        nc.tensor.matmul(
                            state['ps'][:, :CH],
                            xT_sb[:, o, i * P:(i + 1) * P],
                            wvT_sb[:, o, :],
                            start=(o == 0), stop=(o == DB - 1))
                    return step

                def tail():
                    nc.vector.tensor_copy(
                        vaug[:, i, :, 0:HD],
                        state['ps'][:, :CH].rearrange("p (h d) -> p h d", h=HPC))

                return [mk_mm(o) for o in range(DB)] + [tail]

            def wo_unit(i, cb_major=False):
                """One 128-row t-block of the output projection:
                2 j-halves x 2 cb-accumulation MMs, one [128, D] DMA out.
                cb_major orders both cb0 MMs first (tail: cb1 waits on the
                last normalize)."""
                state = {}

                def mk_mm(j, cb):
                    def step():
                        if cb == 0:
                            state[j] = pspool.tile(
                                [P, NQ], F32, tag="b1", bufs=4, name=f"o_{i}_{j}")
                        nc.tensor.matmul(
                            state[j][:],
                            attnT_sb[:, cb, i * P:(i + 1) * P],
                            woT_sb[:, cb, j * NQ:(j + 1) * NQ],
                            start=(cb == 0), stop=(cb == CB - 1))
                        if cb == CB - 1:
                            if 'ob' not in state:
                                state['ob'] = obp.tile([P, D], F16, tag="ob",
                                                       name=f"ob_{i}")
                            nc.vector.tensor_copy(
                                state['ob'][:, j * NQ:(j + 1) * NQ],
                                state[j][:])
                    return step

                def tail():
                    nc.gpsimd.dma_start(out[i * P:(i + 1) * P, :],
                                        state['ob'][:])

                order = ([(j, cb) for cb in range(CB) for j in range(D // NQ)]
                         if cb_major else
                         [(j, cb) for j in range(D // NQ) for cb in range(CB)])
                return [mk_mm(j, cb) for j, cb in order] + [tail]

            class StepQueue:
                def __init__(self):
                    self.steps = []

                def add_units(self, units):
                    for u in units:
                        self.steps.extend(u)

                def pop(self, n=1):
                    for _ in range(n):
                        if self.steps:
                            self.steps.pop(0)()

                def pop_even(self, iters_left):
                    # drain the queue evenly over the remaining iterations
                    n = -(-len(self.steps) // max(iters_left, 1))
                    self.pop(min(n, 4))

                def flush(self):
                    while self.steps:
                        self.steps.pop(0)()

            def kb_list(qc):
                return list(range(min(NKB, (qc + 1) * (NQ // P)))) if causal \
                    else list(range(NKB))

            # ---------------- attention ----------------
            # `pend` threads the one-block exp/PV lookahead ACROSS calls so
            # the PE stream has no bubble at q-chunk / head-pair boundaries
            pend_box = [None]

            def attention(hp, qc, fq, pre_iter=None, on_start=None):
                kbs = kb_list(qc)
                q0 = qc * NQ
                otps = [pspool.tile([HD + 1, NQ], F32, tag="b1", bufs=4,
                                    name=f"ot_{hp}_{qc}_{i}")
                        for i in range(2)]

                def finish(kb, stp2, qsl):
                    pt = ptp.tile([P, 2, NQ], F16, tag="pt",
                                  name=f"pt_{hp}_{qc}_{kb}")
                    sflat = stp2.rearrange("p a b -> p (a b)")
                    pflat = pt.rearrange("p a b -> p (a b)")
                    # one exp covers both halves; the uncomputed middle columns
                    # of diagonal blocks are never read downstream
                    nc.scalar.activation(pflat[:, qsl:2 * NQ],
                                         sflat[:, qsl:2 * NQ],
                                         AF.Exp, scale=float(HD) ** -0.5)
                    for half in range(2):
                        h = hp * 2 + half
                        nc.tensor.matmul(
                            otps[half][:, qsl:NQ],
                            vaug[:, kb, h, :],
                            pt[:, half, qsl:NQ],
                            start=(kb == kbs[0]), stop=(kb == kbs[-1]))

                for n_it, kb in enumerate(kbs):
                    qsl = max(0, kb * P - q0) if causal else 0
                    diag = causal and kb * P >= q0
                    stp2 = pspool.tile([P, 2, NQ], F32, tag="st",
                                       name=f"st_{hp}_{qc}_{kb}")
                    # both halves' score matmuls back-to-back so the two
                    # K=64 row-groups run concurrently; masks after
                    for half in range(2):
                        hb = half * HD
                        nc.tensor.matmul(
                            stp2[:, half, qsl:NQ],
                            KT_sb[hb:hb + HD, hp, kb * P:(kb + 1) * P],
                            QT_sb[hb:hb + HD, hp, q0 + qsl:q0 + NQ],
                            start=True, stop=not diag)
                    if diag:
                        # causal mask: add -30000 strictly below the
                        # diagonal so exp underflows those to zero
                        for half in range(2):
                            nc.tensor.matmul(
                                stp2[:, half, qsl:qsl + P],
                                ident_sb[:],
                                triB_sb[:],
                                start=False, stop=True)
                    # emit the q-chunk's V rows after its scores so exp can
                    # start as early as possible on the first chunk
                    if pre_iter is not None:
                        pre_iter(kb)
                    fq.pop_even(len(kbs) - n_it)
                    if pend_box[0] is not None:
                        pf, args = pend_box[0]
                        pf(*args)
                    pend_box[0] = (finish, (kb, stp2, qsl))
                    if n_it == 0 and on_start is not None:
                        # previous call's stage/normalize, emitted after this
                        # call's first scores so the PE stream has no bubble
                        on_start()
                return otps

            def drain_pend():
                if pend_box[0] is not None:
                    pf, args = pend_box[0]
                    pf(*args)
                    pend_box[0] = None

            def stage(hp, qc, otps):
                # drain PSUM into the staged SBUF buffer (frees the banks);
                # runs on the scalar engine (Copy is in the exp table set and
                # its input is always ready, so the exp stream never stalls)
                for half in range(2):
                    nc.scalar.activation(otsb[:, hp, qc * 2 + half, :],
                                         otps[half][:], AF.Copy)

            def normalize(qc, hps):
                # pack the sums rows onto partitions via a DRAM bounce, one
                # batched reciprocal (DVE reciprocal cost is per-column),
                # broadcast back across partitions with stride-0 DRAM row
                # re-reads (the baseline-proven pattern), then multiply
                rows = [(hp, half) for hp in hps for half in range(2)]
                nr = len(rows)
                r0 = hps[0] * 2
                for r, (hp, half) in enumerate(rows):
                    nc.sync.dma_start(sumd[qc, r0 + r],
                                      otsb[HD:HD + 1, hp, qc * 2 + half, :])
                sums4 = ssm.tile([nr, NQ], F32, tag="s4",
                                 name=f"s4_{qc}_{hps[0]}")
                nc.sync.dma_start(sums4[:], sumd[qc, r0:r0 + nr])
                rec4 = ssm.tile([nr, NQ], F32, tag="r4",
                                name=f"r4_{qc}_{hps[0]}")
                nc.vector.reciprocal(rec4[:], sums4[:])
                nc.sync.dma_start(recd[qc, r0:r0 + nr], rec4[:])
                for r, (hp, half) in enumerate(rows):
                    bc = bcp.tile([HD, NQ], F32, tag="bc",
                                  name=f"bc_{qc}_{hp}_{half}")
                    row = recd[qc, r0 + r]
                    src = bass.AP(tensor=row.tensor, offset=row.offset,
                                  ap=[[0, HD]] + list(row.ap))
                    nc.sync.dma_start(bc[:], src)
                    nc.vector.tensor_tensor(
                        attnT_sb[half * HD:(half + 1) * HD, hp,
                                 qc * NQ:(qc + 1) * NQ],
                        otsb[0:HD, hp, qc * 2 + half, :], bc[:], MUL)

            # ---------------- emission schedule ----------------
            fq = StepQueue()

            # prologue: q/k cb0 m0 projections (att(0,0)'s V rows are emitted
            # inside the kb loop so exp starts as early as possible)
            fq.add_units([proj_unit(wqT_sb, QT_sb, 0, 0, "q"),
                          proj_unit(wkT_sb, KT_sb, 0, 0, "k")])
            fq.flush()

            prev = {}
            for qc in range(NT):
                # fillers for att(0,qc): cb1 projections of this chunk (needed
                # by att(1,qc)), then the wo tiles of the previous chunk
                # (chunk NT-2's wo units are all reserved for the tail)
                fq.add_units([proj_unit(wqT_sb, QT_sb, 1, qc, "q"),
                              proj_unit(wkT_sb, KT_sb, 1, qc, "k")])
                if 0 < qc < NT - 1:
                    fq.add_units([wo_unit(i)
                                  for i in range(4 * (qc - 1), 4 * qc)])

                # att(0,0): V row kb must be resident before finish(kb) reads
                # it, so emit each vproj unit inline at the top of iteration kb
                def pre0(kb):
                    for step in vproj_unit(kb):
                        step()

                def on_start0(qc=qc):
                    stage(1, qc - 1, prev['ots1'])
                    normalize(qc - 1, [0, 1])

                ots0 = attention(0, qc, fq,
                                 pre_iter=pre0 if qc == 0 else None,
                                 on_start=on_start0 if qc > 0 else None)
                fq.flush()

                # fillers for att(1,qc): next chunk's cb0 projections + V rows
                if qc + 1 < NT:
                    fq.add_units([proj_unit(wqT_sb, QT_sb, 0, qc + 1, "q"),
                                  proj_unit(wkT_sb, KT_sb, 0, qc + 1, "k")])
                    fq.add_units([vproj_unit(i)
                                  for i in range(4 * (qc + 1), 4 * (qc + 2))])

                def on_start1(qc=qc, ots0=ots0):
                    stage(0, qc, ots0)
                    if qc == NT - 1:
                        # last chunk: normalize hp0 early so only hp1's chain
                        # remains on the critical path at the very end
                        normalize(qc, [0])

                ots1 = attention(1, qc, fq, on_start=on_start1)
                fq.flush()
                prev['ots1'] = ots1

            drain_pend()
            stage(1, NT - 1, prev['ots1'])
            # tail fast path for hp1 of the last chunk: reciprocals read the
            # sums rows straight from PSUM (no wait on staging); the
            # multiplies use the staged copy so the banks free early
            t_rs = []
            for half in range(2):
                rs = ssm.tile([1, NQ], F32, tag="rs", name=f"rs_t_{half}")
                nc.vector.reciprocal(rs[:], prev['ots1'][half][HD:HD + 1, :])
                t_rs.append(rs)
            for half in range(2):
                bc = bcp.tile([HD, NQ], F32, tag="bc", name=f"bc_t_{half}")
                nc.gpsimd.partition_broadcast(bc[:], t_rs[half][:])
                nc.vector.tensor_tensor(
                    attnT_sb[half * HD:(half + 1) * HD, 1,
                             (NT - 1) * NQ:NT * NQ],
                    otsb[0:HD, 1, 2 * NT - 2 + half, :], bc[:], MUL)

            # tail: the reserved chunk-NT-2 units (no dependency on the final
            # normalize) interleaved with the last chunk's tiles, cb0 matmuls
            # first (they only need hp0's columns, normalized early)
            tail_units = ([wo_unit(i) for i in range(4 * (NT - 2),
                                                     4 * (NT - 1))] +
                          [wo_unit(i, cb_major=True)
                           for i in range(4 * (NT - 1), NKB)])
            for ua, ub in zip(tail_units[0::2], tail_units[1::2]):
                for sa, sb in zip(ua, ub):
                    sa()
                    sb()

    nc.compile()
    return nc


def _get_program(causal: bool):
    key = ("causal" if causal else "full")
    if key not in _prog_cache:
        _prog_cache[key] = _build_program(causal)
    return _prog_cache[key]


def _mask_kind(mask):
    m = np.asarray(mask)
    if m.ndim == 4:
        m = m[0, 0]
    if (m != 0).all():
        return False  # full attention
    trilm = np.tril(np.ones((m.shape[0], m.shape[1]), dtype=m.dtype))
    if np.array_equal(m, trilm):
        return True
    raise NotImplementedError("mask is neither all-ones nor causal tril")


def _make_in_maps(x, cos, sin, wq, wk, wv, wo):
    x = np.asarray(x, dtype=np.float32)
    cos = np.asarray(cos, dtype=np.float32)
    sin = np.asarray(sin, dtype=np.float32)
    wq = np.asarray(wq, dtype=np.float32)
    wk = np.asarray(wk, dtype=np.float32)
    wv = np.asarray(wv, dtype=np.float32)
    wo = np.asarray(wo, dtype=np.float32)

    # RoPE tables in transposed head-pair layout [128ch, T].
    # cosS[c, t] = cos[t, c % 64]; sinS flips sign on the low half of each head
    # (rotate_half's minus), matching qswp[p] = q[p^32] on the device.
    ci = np.arange(P) % HD
    cosS = np.ascontiguousarray(cos[:T, ci].T.astype(np.float16))   # [128, T]
    sgn = np.where((np.arange(P) % HD) < (HD // 2), -1.0, 1.0).astype(np.float32)
    sinSm = np.ascontiguousarray(
        (sin[:T, ci].T * sgn[:, None]).astype(np.float16))          # [128, T]
    identm = np.eye(P, dtype=np.float16)
    triBm = np.ascontiguousarray(
        (np.tril(np.ones((P, P), np.float32), -1) * -30000.0).astype(np.float16))
    ones = np.ones((P, (T // P) * HPC), dtype=np.float16)

    in_maps = []
    for core in range(NCORES):
        b = core // GROUPS
        g = core % GROUPS
        c0 = g * CH
        in_maps.append({
            "xT": np.ascontiguousarray(x[b].T.astype(np.float16)),          # [D, T]
            "wqT": np.ascontiguousarray(wq[c0:c0 + CH, :].T.astype(np.float16)),
            "wkT": np.ascontiguousarray(wk[c0:c0 + CH, :].T.astype(np.float16)),
            "wvT": np.ascontiguousarray(wv[c0:c0 + CH, :].T.astype(np.float16)),
            "woT": np.ascontiguousarray(wo[:, c0:c0 + CH].T.astype(np.float16)),
            "cosS": cosS,
            "sinS": sinSm,
            "ident": identm,
            "triB": triBm,
            "onescol": ones,
        })
    return in_maps


def _run(inputs, trace=False):
    from concourse import bass_utils
    causal = _mask_kind(inputs["mask"])
    nc = _get_program(causal)
    in_maps = _make_in_maps(
        inputs["x"], inputs["cos"], inputs["sin"],
        inputs["wq"], inputs["wk"], inputs["wv"], inputs["wo"])
    if trace:
        _install_ntff_shim()
    res = bass_utils.run_bass_kernel_spmd(
        nc, in_maps, core_ids=list(range(NCORES)), trace=trace)
    outs = [r["out"] for r in res.results]
    full = np.empty((B, T, D), dtype=np.float32)
    for b in range(B):
        full[b] = outs[b * GROUPS].astype(np.float32)
        for g in range(1, GROUPS):
            full[b] += outs[b * GROUPS + g].astype(np.float32)
    return full, res


def kernel(**inputs):
    full, _ = _run(inputs, trace=False)
    return full


def kernel_profiled(**inputs):
    """Like kernel() but with NTFF tracing; returns (out, BassKernelResults)."""
    return _run(inputs, trace=True)
